# revision 18
# baseline (speedup 1.0000x reference)
"""DGCNN backbone Trainium2 Bass kernel.

Data-parallel over batch: B=8 samples -> 8 NeuronCores, one sample per core.
Weights replicated (inference only, no grads -> no collectives).

Per-core pipeline for each EdgeConv layer (N=2048 points, k=10):
  - D[i,j] = 2<x_i,x_j> - |x_j|^2  (row-constant |x_i|^2 dropped; ranking
    equivalent to the reference's negative squared distance) via PE matmuls.
  - exact top-10 per row on the vector engine: max8 -> max_index -> match
    replace(-inf) -> max8 -> max_index  (two rounds of 8, take 8 + 2).
  - P[j] = s * (Wn x_j), Q[i] = s * ((Wc - Wn) x_i) + t   (BN affine folded
    into the weights host-side; LeakyReLU is monotone so
    max_k lrelu(P[j]+Q[i]) = lrelu(max_k P[j] + Q[i]), including sign
    handling because P carries the full signed scale s).
  - P table -> DRAM; ten indirect DMA gathers (one per neighbor slot) with
    inline CCE max accumulation compute max_k P[j] with no engine cost.
  - out = lrelu(G + Q); transpose to channel-major via PE for the next layer.
Head: per-channel global max over points, folded linear+BN, LeakyReLU.
"""

import numpy as np

EPS = 1e-5
NEG_SLOPE = 0.2
B = 8
N = 2048
K = 10
CHANS = [(5, 64), (64, 64), (64, 128), (128, 256)]
NEG_INF = -3.0e38

_CACHE = {}


def _build(n_pts):
    import concourse.bass as bass
    import concourse.mybir as mybir
    from concourse import bacc
    from concourse.bass import IndirectOffsetOnAxis
    from concourse.masks import make_identity
    from concourse.tile import TileContext

    f32 = mybir.dt.float32
    u32 = mybir.dt.uint32
    Alu = mybir.AluOpType
    Act = mybir.ActivationFunctionType

    n_tiles = n_pts // 128
    nc = bacc.Bacc("TRN2", dynamic_dma_scratch_size=32768)

    # ---- I/O ----
    xT_in = nc.dram_tensor("xT", [CHANS[0][0], n_pts], f32, kind="ExternalInput")
    w_in = {}
    for li, (ci, co) in enumerate(CHANS, start=1):
        w_in[f"wp{li}T"] = nc.dram_tensor(f"wp{li}T", [ci, co], f32, kind="ExternalInput")
        w_in[f"wq{li}T"] = nc.dram_tensor(f"wq{li}T", [ci, co], f32, kind="ExternalInput")
        w_in[f"tb{li}"] = nc.dram_tensor(f"tb{li}", [1, co], f32, kind="ExternalInput")
    w5T_in = nc.dram_tensor("w5T", [128, 5, 1024], f32, kind="ExternalInput")
    b5_in = nc.dram_tensor("b5r", [1, 1024], f32, kind="ExternalInput")
    out_g = nc.dram_tensor("out_g", [1, 1024], f32, kind="ExternalOutput")
    out_x4t = nc.dram_tensor("out_x4t", [256, n_pts], f32, kind="ExternalOutput")

    with TileContext(nc) as tc:
        with (
            tc.tile_pool(name="persist", bufs=1) as pp,
            tc.tile_pool(name="work", bufs=2) as wp,
            tc.tile_pool(name="psum", bufs=1, space="PSUM") as psp,
            tc.tile_pool(name="psum2", bufs=1, space="PSUM") as psp2,
            tc.tile_pool(name="psumt", bufs=2, space="PSUM") as pst,
            tc.tile_pool(name="dram", bufs=1, space="DRAM") as dp,
        ):
            ident = pp.tile([128, 128], f32, tag="ident")
            make_identity(nc, ident[:])
            ones_c = pp.tile([128, 1], f32, tag="ones_c")
            nc.vector.memset(ones_c[:], 1.0)
            ones_r = pp.tile([1, 128], f32, tag="ones_r")
            nc.vector.memset(ones_r[:], 1.0)
            one11 = pp.tile([1, 1], f32, tag="one11")
            nc.vector.memset(one11[:], 1.0)
            xg = pp.tile([128, 5], f32, tag="xg")
            nc.vector.memset(xg[:], 0.0)
            i16 = mybir.dt.int16
            i32 = mybir.dt.int32
            # repl[q, p] = 1.0 if p % 16 == q  (16 x 128): lhsT for the
            # broadcast matmul that replicates a 16-row wrapped index block
            # to all 128 partitions (Q7 cores each read their own 16).
            iota_qp = pp.tile([16, 128], i32, tag="iota_qp")
            nc.gpsimd.iota(iota_qp[:], pattern=[[1, 128]], base=0,
                           channel_multiplier=0)
            nc.vector.tensor_scalar(iota_qp[:], iota_qp[:], 15, None,
                                    op0=Alu.bitwise_and)
            qcol = pp.tile([16, 1], i32, tag="qcol")
            nc.gpsimd.iota(qcol[:], pattern=[[0, 1]], base=0,
                           channel_multiplier=1)
            qcolf = pp.tile([16, 1], f32, tag="qcolf")
            nc.vector.tensor_copy(qcolf[:], qcol[:])
            iota_qpf = pp.tile([16, 128], f32, tag="iota_qpf")
            nc.vector.tensor_copy(iota_qpf[:], iota_qp[:])
            repl = pp.tile([16, 128], f32, tag="repl")
            nc.vector.tensor_scalar(repl[:], iota_qpf[:], qcolf[:, :], None,
                                    op0=Alu.is_equal)

            # load layer weights
            wsb = {}
            for li, (ci, co) in enumerate(CHANS, start=1):
                for nm, shp in ((f"wp{li}T", [ci, co]), (f"wq{li}T", [ci, co]),
                                (f"tb{li}", [1, co])):
                    t = pp.tile(shp, f32, tag=nm)
                    nc.sync.dma_start(t[:], w_in[nm][:])
                    wsb[nm] = t

            # x^T  (C x N), layer-1 input
            xt = pp.tile([CHANS[0][0], n_pts], f32, tag="xt1")
            nc.sync.dma_start(xt[:], xT_in[:])

            xg_cols = []  # (tile, rows, col) for the global max step
            x4t_blocks = None

            for li, (ci, co) in enumerate(CHANS, start=1):
                wpT, wqT, tb = wsb[f"wp{li}T"], wsb[f"wq{li}T"], wsb[f"tb{li}"]

                # ranking matrix: inner - |x_j|^2/2  (monotone-equivalent
                # to the reference's 2*inner - |x_i|^2 - |x_j|^2)
                sq = pp.tile([ci, n_pts], f32, tag="sq")
                nc.vector.tensor_mul(sq[:], xt[:], xt[:])
                jslices = [slice(a, min(a + 512, n_pts)) for a in range(0, n_pts, 512)]
                xx_ps = psp.tile([1, n_pts], f32, tag="dps")
                for s in jslices:
                    nc.tensor.matmul(xx_ps[:, s], ones_c[:ci, :], sq[:, s],
                                     start=True, stop=True)
                negxx = pp.tile([1, n_pts], f32, tag="negxx")
                nc.scalar.activation(negxx[:], xx_ps[:], Act.Copy, bias=0.0, scale=-0.5)

                idx_all = pp.tile([128, K, n_tiles], u32, tag="idx")
                q_all = pp.tile([128, n_tiles, co], f32, tag="qall")
                p_dram = dp.tile([n_pts, co], f32, tag=f"ptab{li}")

                for t in range(n_tiles):
                    ts = slice(t * 128, (t + 1) * 128)
                    lhs = xt[:, ts]  # C x 128 stationary
                    # ---- distance tile ----
                    d_ps = psp.tile([128, n_pts], f32, tag="dps")
                    for s in jslices:
                        nc.tensor.matmul(d_ps[:, s], lhs, xt[:, s],
                                         start=True, stop=False)
                        nc.tensor.matmul(d_ps[:, s], ones_r[:], negxx[:, s],
                                         start=False, stop=True)
                    d_sb = wp.tile([128, n_pts], f32, tag="dsb")
                    nc.scalar.activation(d_sb[:], d_ps[:], Act.Copy, bias=0.0, scale=1.0)

                    # ---- exact top-10 per row ----
                    v8 = wp.tile([128, 8], f32, tag="v8")
                    i8b = wp.tile([128, 8], u32, tag="i8b")
                    nc.vector.max(out=v8[:], in_=d_sb[:])
                    nc.vector.max_index(idx_all[:, 0:8, t], v8[:], d_sb[:])
                    nc.vector.match_replace(out=d_sb[:], in_to_replace=v8[:],
                                            in_values=d_sb[:], imm_value=NEG_INF)
                    nc.vector.max(out=v8[:], in_=d_sb[:])
                    nc.vector.max_index(i8b[:], v8[:], d_sb[:])
                    nc.vector.tensor_copy(idx_all[:, 8:10, t], i8b[:, 0:2])

                    # ---- P and Q tiles ----
                    p_ps = psp2.tile([128, co], f32, tag="pps")
                    nc.tensor.matmul(p_ps[:], lhs, wpT[:], start=True, stop=True)
                    p_sb = wp.tile([128, co], f32, tag="psb")
                    nc.scalar.activation(p_sb[:], p_ps[:], Act.Copy, bias=0.0, scale=1.0)
                    nc.sync.dma_start(p_dram[ts, :], p_sb[:])

                    q_ps = psp2.tile([128, co], f32, tag="qps")
                    nc.tensor.matmul(q_ps[:], lhs, wqT[:], start=True, stop=False)
                    nc.tensor.matmul(q_ps[:], ones_r[:], tb[:], start=False, stop=True)
                    nc.scalar.activation(q_all[:, t, :], q_ps[:], Act.Copy,
                                         bias=0.0, scale=1.0)

                # ---- wrap indices into dma_gather layout ----
                # list[i] = neighbor idx of point i.  dma_gather wants
                # int16 indices wrapped (partition i%16, column i//16),
                # replicated to all 8 Q7 core blocks.  Route: PE transpose
                # (point-major to p-inner) -> DRAM flat list -> strided
                # clean re-read -> PE transpose -> 0/1 replication matmul.
                idxf = wp.tile([128, K, n_tiles], f32, tag="idxf", name="idxf")
                nc.vector.tensor_copy(idxf[:], idx_all[:])
                l_dram = dp.tile([K, n_pts], f32, tag="ldram", name="ldram")
                for half in range(2):
                    ks = slice(half * (K // 2), (half + 1) * (K // 2))
                    nkt = (K // 2) * n_tiles
                    t1_ps = pst.tile([128, 128], f32, tag="tps", name="t1ps")
                    nc.tensor.transpose(t1_ps[:nkt, :], idxf[:, ks, :], ident[:])
                    t1_sb = wp.tile([nkt, 128], f32, tag="t1sb", name="t1sb")
                    nc.scalar.activation(t1_sb[:], t1_ps[:nkt, :], Act.Copy,
                                         bias=0.0, scale=1.0)
                    nc.sync.dma_start(l_dram[ks, :], t1_sb[:])
                nq = n_pts // 16
                v_all = wp.tile([nq, K, 16], f32, tag="vall", name="vall")
                nc.sync.dma_start(
                    v_all[:], l_dram[:].rearrange("k (c q) -> c k q", q=16))
                widx = pp.tile([128, K, nq], i16, tag="widx")
                for kk in range(K):
                    w16_ps = pst.tile([128, 128], f32, tag="tps", name="w16ps")
                    nc.tensor.transpose(w16_ps[:16, :nq], v_all[:, kk, :], ident[:nq, :nq])
                    w16_sb = wp.tile([16, nq], f32, tag="w16sb", name="w16sb")
                    nc.scalar.activation(w16_sb[:], w16_ps[:16, :nq], Act.Copy,
                                         bias=0.0, scale=1.0)
                    w_ps = pst.tile([128, 128], f32, tag="tps", name="wps")
                    nc.tensor.matmul(w_ps[:, :nq], repl[:, :], w16_sb[:, :],
                                     start=True, stop=True)
                    nc.vector.tensor_copy(widx[:, kk, :], w_ps[:, :nq])

                # ---- gather P rows per neighbor slot, max-reduce over k ----
                CHUNK = min(1024, n_pts)
                nchunk = n_pts // CHUNK
                reg_chunk = nc.gpsimd.to_reg(CHUNK)
                g = pp.tile([128, n_tiles, co], f32, tag="g")
                for kk in range(K):
                    dst = g if kk == 0 else wp.tile([128, n_tiles, co], f32,
                                                    tag="gbuf", name="gbuf")
                    for cc in range(nchunk):
                        nc.gpsimd.dma_gather(
                            out_ap=dst[:, cc * (CHUNK // 128):(cc + 1) * (CHUNK // 128), :],
                            in_ap=p_dram[:, :],
                            idxs_ap=widx[:, kk, cc * (CHUNK // 16):(cc + 1) * (CHUNK // 16)],
                            num_idxs=CHUNK, num_idxs_reg=reg_chunk,
                            elem_size=co)
                    if kk > 0:
                        nc.vector.tensor_tensor(g[:], g[:], dst[:], op=Alu.max)

                # ---- out = lrelu(G + Q), then transpose to channel-major ----
                nc.vector.tensor_add(g[:], g[:], q_all[:])
                # lrelu(x) = max(0.2*x, x), in place
                xo = g
                nc.vector.scalar_tensor_tensor(
                    out=xo[:], in0=g[:], scalar=NEG_SLOPE, in1=g[:],
                    op0=Alu.mult, op1=Alu.max)

                n_blk = (co + 127) // 128
                xt_next = [pp.tile([min(128, co - b * 128), n_pts], f32,
                                   tag=f"xtn{li % 2}_{b}", name=f"xtn{li}_{b}")
                           for b in range(n_blk)]
                for t in range(n_tiles):
                    for b in range(n_blk):
                        bs = slice(b * 128, min(co, (b + 1) * 128))
                        nb = bs.stop - bs.start
                        t_ps = pst.tile([128, 128], f32, tag="tps")
                        nc.tensor.transpose(t_ps[:nb, :], xo[:, t, bs], ident[:])
                        nc.scalar.activation(
                            xt_next[b][:, t * 128:(t + 1) * 128], t_ps[:nb, :],
                            Act.Copy, bias=0.0, scale=1.0)

                # global max contribution of this layer
                col0 = {1: 0, 2: 1, 3: 2, 4: 3}[li]
                for b in range(n_blk):
                    nco = xt_next[b].shape[0]
                    nc.vector.reduce_max(xg[0:nco, col0 + b:col0 + b + 1],
                                         xt_next[b][:, :], axis=mybir.AxisListType.X)

                if li == 4:
                    x4t_blocks = xt_next
                else:
                    xt = xt_next[0]

            # ---- head: h = W5' @ xg + b5', lrelu ----
            w5 = pp.tile([128, 5, 1024], f32, tag="qall", name="w5")
            nc.sync.dma_start(w5[:], w5T_in[:])
            b5 = pp.tile([1, 1024], f32, tag="sq", name="b5")
            nc.sync.dma_start(b5[:], b5_in[:])
            h_ps = psp.tile([1, 1024], f32, tag="dps")
            for nch in range(2):
                s = slice(nch * 512, (nch + 1) * 512)
                for j in range(5):
                    nc.tensor.matmul(h_ps[:, s], xg[:, j:j + 1], w5[:, j, s],
                                     start=(j == 0), stop=False)
                nc.tensor.matmul(h_ps[:, s], one11[:], b5[:, s],
                                 start=False, stop=True)
            h_sb0 = pp.tile([1, 1024], f32, tag="negxx", name="h_sb0")
            nc.scalar.activation(h_sb0[:], h_ps[:], Act.Copy, bias=0.0, scale=1.0)
            h_sb = pp.tile([1, 1024], f32, tag="idx", name="h_sb")
            nc.vector.scalar_tensor_tensor(
                out=h_sb[:], in0=h_sb0[:], scalar=NEG_SLOPE, in1=h_sb0[:],
                op0=Alu.mult, op1=Alu.max)
            nc.sync.dma_start(out_g[:], h_sb[:])
            for b in range(2):
                nc.sync.dma_start(out_x4t[b * 128:(b + 1) * 128, :], x4t_blocks[b][:])

    nc.compile()
    return nc


def _host_prep(inputs, n_pts):
    """Fold BN into weights; build per-core input maps."""
    inp = {k: np.asarray(v, dtype=np.float32) for k, v in inputs.items()}
    maps_common = {}
    for li, (ci, co) in enumerate(CHANS, start=1):
        w = inp[f"w{li}"]          # (co, 2ci)
        g, b = inp[f"g{li}"], inp[f"b{li}"]
        rm, rv = inp[f"rm{li}"], inp[f"rv{li}"]
        s = g / np.sqrt(rv + EPS)
        t = b - rm * s
        wn, wc = w[:, :ci], w[:, ci:]
        maps_common[f"wp{li}T"] = np.ascontiguousarray((s[:, None] * wn).T)
        maps_common[f"wq{li}T"] = np.ascontiguousarray((s[:, None] * (wc - wn)).T)
        maps_common[f"tb{li}"] = t.reshape(1, co)
    s5 = inp["g5"] / np.sqrt(inp["rv5"] + EPS)
    t5 = inp["b5"] - inp["rm5"] * s5
    w5eff = s5[:, None] * inp["lin_w"]          # (1024, 512)
    b5eff = (s5 * inp["lin_b"] + t5).reshape(1, 1024)
    # xg layout: col0=x1(rows 0:64), col1=x2(0:64), col2=x3(0:128),
    # col3=x4[0:128], col4=x4[128:256]
    w5T = np.zeros((128, 5, 1024), np.float32)
    w5T[0:64, 0, :] = w5eff[:, 0:64].T
    w5T[0:64, 1, :] = w5eff[:, 64:128].T
    w5T[0:128, 2, :] = w5eff[:, 128:256].T
    w5T[0:128, 3, :] = w5eff[:, 256:384].T
    w5T[0:128, 4, :] = w5eff[:, 384:512].T
    maps_common["w5T"] = w5T
    maps_common["b5r"] = b5eff

    x = inp["x"]  # (B, N, 5)
    in_maps = []
    for bb in range(x.shape[0]):
        m = dict(maps_common)
        m["xT"] = np.ascontiguousarray(x[bb, :n_pts, :].T)
        in_maps.append(m)
    return in_maps


def kernel(**inputs):
    from concourse.bass_utils import run_bass_kernel_spmd

    key = ("prog", N)
    if key not in _CACHE:
        _CACHE[key] = _build(N)
    nc = _CACHE[key]

    in_maps = _host_prep(inputs, N)
    res = run_bass_kernel_spmd(nc, in_maps, core_ids=list(range(B)))
    global LAST_RESULTS
    LAST_RESULTS = res
    xg = np.stack([r["out_g"].reshape(1024) for r in res.results])
    x4t = np.stack([r["out_x4t"] for r in res.results])
    return xg.astype(np.float32), x4t.astype(np.float32)
